# revision 48
# baseline (speedup 1.0000x reference)
"""Trainium2 Bass kernel for the ControlUnit problem.

Computation (per batch b):
    cq      = concat([control_state, question])            # [2D]
    cq_proj = cq @ W_cq + b_cq + step_emb[step]            # [D]
    qw      = cq_proj * W_attn                             # [D]
    logits  = context[b] @ qw  (+ b_attn, softmax-invariant, dropped)
    w       = softmax(where(mask, logits, -1e4))           # [L]
    out[b]  = w @ context[b]                               # [D]

Sharding: data-parallel over batch across 8 NeuronCores (8 batches/core);
params replicated (W_cq cast to bf16 to halve its DMA traffic; a K-sharded
W_cq + ReduceScatter variant was measured slower on this runtime).

Per-core kernel:
  Phase 1: cq_proj matmul in bf16 (bias folded in via an augmented ones
           column of cq / bias row of W; [8,512] PSUM chunk accumulators
           drained to SBUF), qw = cq_proj * W_attn, PE-transpose qw so d
           sits on partitions.
  Phase 2 (per batch): one DMA streams the batch's context [512, D]
           (pre-cast to bf16 on the host, halving the dominant HBM
           stream); PE transposes it into
           [128d, l] chunks (PSUM->SBUF copies split between DVE and ACT);
           a matmul of qwT against the transposed chunks accumulates
           logits, with the -1e4 mask bias added by an extra
           identity-selector matmul; ACT exp (no max subtraction - logits
           are ~N(0,1)); a predicated copy keeps the true softmax-numerator
           row of each batch for the denominators; PE transposes the
           exp-weights into l-on-partition columns (zero-padded per-batch
           weight blocks); the weighted sum accumulates in [8,512] PSUM
           chunks, DVE-added into an SBUF accumulator.  Denominators come
           from one reduce_sum/reciprocal; the final scale + output DMA
           run per 512-chunk so the tail overlaps the last batch.
  Batch 0's load+transposes are emitted before the W_cq stream so the PE
  has fill work while weights stream in; the bf16 constants (identity,
  mask bias, cqT) ride one blob DMA so HWDGE fixed costs don't serialize
  ahead of the first transposes.

All matmul operands are bf16 (fp32 matmul is 4x slower on the PE; fp32
accumulation in PSUM throughout), giving ~0.6% relative error.
"""
import numpy as np
import ml_dtypes
from contextlib import ExitStack

import concourse.bass as bass
import concourse.tile as tile
from concourse import bacc, mybir
from concourse.bass_utils import run_bass_kernel_spmd

F32 = mybir.dt.float32
BF16 = mybir.dt.bfloat16

N_CORES = 8
B, L, D = 64, 512, 2048


def build_nc(b_c, l, d, n_cores, with_mask=True):
    """Build + compile the per-core Bass program (SPMD: same program on all
    cores, different data)."""
    d2a = 2 * d + 128          # augmented contraction dim (bias row block)
    KT = d2a // 128            # k-tiles for the cq_proj matmul
    LT = l // 128              # l-tiles per batch
    DC = d // 128              # 128-wide d-chunks
    NN = d // 512              # 512-wide n-chunks
    CT_G = min(8, DC)          # transposed chunks per PSUM group
    NG = DC // CT_G

    nc = bacc.Bacc("TRN2", target_bir_lowering=False, debug=False,
                   num_devices=n_cores)

    ctx_d = nc.dram_tensor("ctx", [b_c, l, d], BF16, kind="ExternalInput")
    BLOB = 128 + l + KT * b_c
    blob_d = nc.dram_tensor("blob", [128, BLOB], BF16, kind="ExternalInput")
    w_d = nc.dram_tensor("w_aug", [d2a, d], BF16, kind="ExternalInput")
    idf_d = nc.dram_tensor("idf", [b_c, b_c], F32, kind="ExternalInput")
    out_d = nc.dram_tensor("out", [b_c, d], F32, kind="ExternalOutput")

    Exp = mybir.ActivationFunctionType.Exp

    with tile.TileContext(nc) as tc:
        with ExitStack() as ctx:
            const = ctx.enter_context(tc.tile_pool(name="const", bufs=1))
            wpool = ctx.enter_context(tc.tile_pool(name="wpool", bufs=4))
            natpool = ctx.enter_context(tc.tile_pool(name="natpool", bufs=3))
            ctpool = ctx.enter_context(tc.tile_pool(name="ctpool", bufs=3))
            ps_lg_p = ctx.enter_context(tc.tile_pool(name="ps_lg_p", bufs=2, space="PSUM"))
            ps_out_p = ctx.enter_context(tc.tile_pool(name="ps_out_p", bufs=2, space="PSUM"))
            lgpool = ctx.enter_context(tc.tile_pool(name="lgpool", bufs=2))
            ps_ct_p = ctx.enter_context(tc.tile_pool(name="ps_ct_p", bufs=4, space="PSUM"))

            # ---- constants / persistent tiles ----
            # one blob DMA avoids FIFO-serializing ~2us fixed costs on the
            # HWDGE queue before the first transposes can start
            blob_sb = const.tile([128, BLOB], BF16)
            nc.sync.dma_start(blob_sb[:, 0:128], blob_d[:, 0:128])
            idb = blob_sb[:, 0:128]
            mask_sb = blob_sb[:, 128:128 + l]
            cqT_sb = blob_sb[:, 128 + l:]
            idf_sb = const.tile([b_c, b_c], F32)   # f32 identity (qw transposes)
            nc.gpsimd.dma_start(idf_sb[:, :], idf_d[:, :])

            qw_sb = const.tile([b_c, d], F32)      # phase-1 result (== qw)
            qwT_sb = const.tile([128, DC * b_c], BF16)
            W8 = const.tile([128, b_c, LT, b_c], BF16)
            nc.gpsimd.memset(W8[:, :, :, :], 0.0)
            den_sb = const.tile([b_c, b_c], F32)   # exp row-sums per batch
            den2_sb = const.tile([b_c, b_c], F32)  # second half (last batch)
            nc.gpsimd.memset(den2_sb[:, :], 0.0)
            dend_sb = const.tile([b_c, b_c], F32)
            ssum_sb = const.tile([b_c, 1], F32)
            sinv_sb = const.tile([b_c, 1], F32)
            out_sb = const.tile([b_c, d], F32)

            acc_sb = const.tile([b_c, d], F32)

            def load_blob_rest():
                nc.sync.dma_start(blob_sb[:, 128:], blob_d[:, 128:])

            # ---- phase 2: per-batch attention ----
            nat4s = {}
            ctbs = {}

            def load_and_transpose(b):
                nat4 = natpool.tile([128, LT, d], BF16, name="nat4")
                # context is pre-cast bf16 on the host; split per l-tile so
                # each batch's transposes start on its first quarter
                for i in range(LT):
                    nc.sync.dma_start(
                        nat4[:, i, :], ctx_d[b, i * 128:(i + 1) * 128, :])
                nat4s[b] = nat4

                ctb = ctpool.tile([128, DC, l], BF16, name="ctb")
                for i in range(LT):
                    for g in range(NG):
                        pct = ps_ct_p.tile([128, CT_G, 128], BF16, name="pct")
                        for jj in range(CT_G):
                            j = g * CT_G + jj
                            nc.tensor.transpose(
                                pct[:, jj, :],
                                nat4[:, i, j * 128:(j + 1) * 128],
                                idb[:, :],
                            )
                        dst = ctb[:, g * CT_G:(g + 1) * CT_G,
                                  i * 128:(i + 1) * 128]
                        if (i * NG + g) % 2 == 0:
                            nc.vector.tensor_copy(dst, pct[:, :, :])
                        else:
                            nc.scalar.copy(dst, pct[:, :, :])
                ctbs[b] = ctb

            def attend(b):
                nats = [nat4s[b][:, i, :] for i in range(LT)]
                ctb = ctbs[b]
                last = b == b_c - 1
                # logits for all queries vs this batch's context; row b is
                # real.  First matmul adds the -10000 mask bias.  The LAST
                # batch runs in two l-halves so the exp/exp-weight chain
                # overlaps the second half's logits matmuls.
                pm_sb = lgpool.tile([b_c, l], BF16, name="pm_sb")
                halves = ((0, l // 2), (l // 2, l)) if last else ((0, l),)
                for hi, (l0, l1) in enumerate(halves):
                    ps_lg = ps_lg_p.tile([b_c, l1 - l0], F32, name="ps_lg")
                    if with_mask:
                        nc.tensor.matmul(
                            ps_lg[:, :],
                            lhsT=idb[:, 0:b_c],
                            rhs=mask_sb[:, l0:l1],
                            start=True, stop=False,
                        )
                    for j in range(DC):
                        nc.tensor.matmul(
                            ps_lg[:, :],
                            lhsT=qwT_sb[:, j * b_c:(j + 1) * b_c],
                            rhs=ctb[:, j, l0:l1],
                            start=(j == 0 and not with_mask),
                            stop=(j == DC - 1),
                        )
                    # accum_out gives the exp row-sums = denominators free
                    dslot = den_sb if hi == 0 else den2_sb
                    nc.scalar.activation(pm_sb[:, l0:l1], ps_lg[:, :], Exp,
                                         accum_out=dslot[:, b:b + 1])
                    # exp-weights to l-on-partitions, into W8 column b
                    i0, i1 = l0 // 128, l1 // 128
                    pw = ps_ct_p.tile([128, LT, b_c], BF16, name="pct")
                    for i in range(i0, i1):
                        nc.tensor.transpose(
                            pw[:, i, :],
                            pm_sb[0:b_c, i * 128:(i + 1) * 128],
                            idb[0:b_c, 0:b_c],
                        )
                    # ACT, not DVE: the DVE queue is busy with ct drains
                    nc.scalar.copy(W8[:, b, i0:i1, b], pw[:, i0:i1, b])
                if last:
                    # denominators: diag of (den + den2), reduce, reciprocal
                    nc.vector.tensor_add(dend_sb[:, :], den_sb[:, :],
                                         den2_sb[:, :])
                    nc.vector.tensor_mul(dend_sb[:, :], dend_sb[:, :],
                                         idf_sb[:, :])
                    nc.vector.reduce_sum(ssum_sb[:, :], dend_sb[:, :],
                                         axis=mybir.AxisListType.X)
                    nc.vector.reciprocal(sinv_sb[:, :], ssum_sb[:, :])

                # weighted sum: per-batch chunk accumulators drained to SBUF
                for n in range(NN):
                    ps_o = ps_out_p.tile([b_c, 512], F32, name="ps_out")
                    for i in range(LT):
                        nc.tensor.matmul(
                            ps_o[:, :],
                            lhsT=W8[:, b, i, :],
                            rhs=nats[i][:, n * 512:(n + 1) * 512],
                            start=(i == 0), stop=(i == LT - 1),
                        )
                    dstc = acc_sb[:, n * 512:(n + 1) * 512]
                    if b == 0:
                        nc.vector.tensor_copy(dstc, ps_o[:, :])
                    else:
                        nc.vector.tensor_add(dstc, dstc, ps_o[:, :])
                    if b == b_c - 1:
                        # finalize this chunk immediately; alternate ACT/DVE
                        # so the four scales don't serialize on one engine
                        if n % 2 == 0:
                            nc.scalar.activation(
                                out_sb[:, n * 512:(n + 1) * 512], dstc,
                                mybir.ActivationFunctionType.Copy,
                                scale=sinv_sb[:, :])
                        else:
                            nc.vector.tensor_scalar_mul(
                                out_sb[:, n * 512:(n + 1) * 512], dstc,
                                sinv_sb[:, :])
                        # last two chunks ride one DMA: the ~0.6us HWDGE
                        # issue serialization dominates the tail otherwise
                        if n < NN - 2:
                            nc.sync.dma_start(
                                out_d[:, n * 512:(n + 1) * 512],
                                out_sb[:, n * 512:(n + 1) * 512])
                        elif n == NN - 1:
                            nc.sync.dma_start(
                                out_d[:, (NN - 2) * 512:],
                                out_sb[:, (NN - 2) * 512:])

            # PE warm-up: keep the PE busy early so the p-state ramp
            # reaches full speed before the real work
            for w in range(2):
                pwu = ps_ct_p.tile([128, CT_G, 128], BF16, name="pct")
                for jj in range(CT_G):
                    nc.tensor.transpose(pwu[:, jj, :], idb[:, :], idb[:, :])

            load_and_transpose(0)
            load_blob_rest()

            # ---- phase 1: cq_proj = cq_aug @ W_aug ----
            # chunked [b_c, 512] accumulators borrowed from the lg/out psum
            # pools (phase 1 finishes before batch-0 logits need them).
            accs = []
            for n in range(NN):
                pool = ps_lg_p if n % 2 == 0 else ps_out_p
                nm = "ps_lg" if n % 2 == 0 else "ps_out"
                accs.append(pool.tile([b_c, 512], F32, name=nm))
            # W DMA group sizes ramp up so the PE isn't starved at start
            kgs = []
            for g in (1, 1, 2):
                if sum(kgs) < KT:
                    kgs.append(min(g, KT - sum(kgs)))
            while sum(kgs) < KT:
                kgs.append(min(4, KT - sum(kgs)))
            kg = 0
            for gi, kn in enumerate(kgs):
                if 1 <= gi <= 3:
                    fil = ps_ct_p.tile([128, CT_G, 128], BF16, name="pct")
                    for jj in range(CT_G):
                        nc.tensor.transpose(fil[:, jj, :], idb[:, :], idb[:, :])
                wk = wpool.tile([128, 4, d], BF16, name="wk")
                nc.sync.dma_start(
                    wk[:, 0:kn, :],
                    w_d[kg * 128:(kg + kn) * 128, :].rearrange(
                        "(a p) n -> p a n", p=128))
                for ki in range(kn):
                    k = kg + ki
                    for n in range(NN):
                        nc.tensor.matmul(
                            accs[n],
                            lhsT=cqT_sb[:, k * b_c:(k + 1) * b_c],
                            rhs=wk[:, ki, n * 512:(n + 1) * 512],
                            start=(k == 0),
                            stop=(k == KT - 1),
                        )
                kg += kn
            # per-chunk drain + qwT transposes: each 512-wide accumulator
            # drains (DVE/ACT alternating) and its 4 d-chunks transpose
            # immediately, so the first logits matmuls start sooner
            for n in range(NN):
                dst = qw_sb[:, n * 512:(n + 1) * 512]
                if n % 2 == 0:
                    nc.vector.tensor_copy(dst, accs[n])
                else:
                    nc.scalar.copy(dst, accs[n])
                pq = ps_ct_p.tile([128, 4 * b_c], F32, name="pct")
                for jj in range(4):
                    j = 4 * n + jj
                    nc.tensor.transpose(
                        pq[:, jj * b_c:(jj + 1) * b_c],
                        qw_sb[:, j * 128:(j + 1) * 128],
                        idf_sb[:, :],
                    )
                if n % 2 == 0:
                    nc.vector.tensor_copy(
                        qwT_sb[:, 4 * n * b_c:(4 * n + 4) * b_c], pq[:, :])
                else:
                    nc.scalar.copy(
                        qwT_sb[:, 4 * n * b_c:(4 * n + 4) * b_c], pq[:, :])

            for b in range(b_c):
                if b == 0:
                    # (batch 0 load+transposes were emitted before phase 1)
                    pass
                load_and_transpose(b + 1) if b + 1 < b_c else None
                attend(b)

            # (finalize happens per chunk inside the last attend())

    nc.compile()
    return nc


def host_prep(inputs, n_cores, b_c, l, d):
    """Slice/format the full inputs into per-core input maps."""
    step = int(np.asarray(inputs["step"]))
    context = np.asarray(inputs["context"], dtype=np.float32)
    question = np.asarray(inputs["question"], dtype=np.float32)
    control_state = np.asarray(inputs["control_state"], dtype=np.float32)
    q_mask = np.asarray(inputs["q_mask"])
    W_cq = np.asarray(inputs["W_cq"], dtype=np.float32)
    b_cq = np.asarray(inputs["b_cq"], dtype=np.float32)
    step_emb = np.asarray(inputs["step_emb"], dtype=np.float32)
    W_attn = np.asarray(inputs["W_attn"], dtype=np.float32)

    bf16 = ml_dtypes.bfloat16
    d2 = 2 * d
    d2a = d2 + 128
    KT = d2a // 128

    bias = (b_cq + step_emb[step]).astype(np.float32)          # [d]
    cq = np.concatenate([control_state, question], axis=1)     # [B, 2d]
    Bfull = cq.shape[0]
    cq_aug = np.zeros((Bfull, d2a), dtype=np.float32)
    cq_aug[:, :d2] = cq
    cq_aug[:, d2] = 1.0
    # W_attn folded into the weight columns: phase 1 emits qw directly
    W_aug = np.zeros((d2a, d), dtype=np.float32)
    W_aug[:d2] = W_cq
    W_aug[d2] = bias
    W_aug *= W_attn[None, :]
    W_aug_bf16 = W_aug.astype(bf16)

    def maskadd(m):
        out = np.zeros((128, l), dtype=np.float32)
        out[:b_c] = (m.astype(np.float32) - 1.0) * 10000.0
        return out.astype(bf16)

    ident_bf16 = np.eye(128, dtype=bf16)

    in_maps = []
    for c in range(n_cores):
        rows = slice(c * b_c, (c + 1) * b_c)
        cqT = np.ascontiguousarray(
            cq_aug[rows].T.reshape(KT, 128, b_c).transpose(1, 0, 2)
        ).astype(bf16)                                          # [128, KT, b_c]
        blob = np.concatenate(
            [ident_bf16, maskadd(q_mask[rows]),
             cqT.reshape(128, KT * b_c)], axis=1)
        in_maps.append({
            "ctx": np.ascontiguousarray(context[rows]).astype(bf16),
            "blob": blob,
            "w_aug": W_aug_bf16,
            "idf": np.eye(b_c, dtype=np.float32),
        })
    return in_maps


_NC_CACHE = {}


def _get_nc(b_c, l, d, n_cores, with_mask=True):
    key = (b_c, l, d, n_cores, with_mask)
    if key not in _NC_CACHE:
        _NC_CACHE[key] = build_nc(b_c, l, d, n_cores, with_mask)
    return _NC_CACHE[key]


def kernel(**inputs) -> np.ndarray:
    context = np.asarray(inputs["context"])
    Bfull, l, d = context.shape
    n_cores = N_CORES
    b_c = Bfull // n_cores

    with_mask = not bool(np.asarray(inputs["q_mask"]).all())
    nc = _get_nc(b_c, l, d, n_cores, with_mask)
    in_maps = host_prep(inputs, n_cores, b_c, l, d)
    res = run_bass_kernel_spmd(nc, in_maps, list(range(n_cores)))
    out = np.concatenate([res.results[c]["out"] for c in range(n_cores)], axis=0)
    return out.astype(np.float32)

